# revision 1
# baseline (speedup 1.0000x reference)
"""Trainium2 Bass kernel for dilated sliding-window attention (AttnWrapper).

Reference computation (all fp32):
  combined = [begin | main | end]                       # [8256, 768]
  keys[t]  = combined[t + 32 + off], off in +-{4..32}   # 16 dilated window keys
  q = (main @ wq.T + bq) * 96**-0.5
  k/v = keys @ w{k,v}.T + b{k,v}
  attn = softmax(q.k), ctx = attn.v, out = [main | ctx @ wo.T + bo]

Sharding: tokens across 8 cores (1024 each) with a 64-row halo of the
combined buffer; weights replicated. Each core computes attn_outT
[768, 1024]; the host transposes and concatenates with main.

Device-side math notes:
 - bk dropped (softmax shift invariance); bv folded into bo' = wo@bv + bo.
 - q scale and bq folded into wq/bq on the host.
 - Matmuls run as float32r (TF32-like, ~1.5e-4 rel err, 4x faster than fp32).
 - Attention probabilities and V are bf16 (A,V quantization ~2e-3 on ctx).
 - Scores are computed transposed (S[key, token]) in groups of 256 tokens x
   320 keys split into key-chunks of (128,128,64); only the valid token
   window of each chunk (128/192/64 wide) is exp'd/masked/matmul'd.
 - A ones-column appended to each V head block makes the ctx matmul also
   produce the softmax denominator (row 96 of the ctx PSUM tile).
"""

import numpy as np

EMBED_DIM = 768
NUM_HEADS = 8
HEAD_DIM = 96
OVERLAP = 32
HALO = 2 * OVERLAP          # 64 extra combined rows per core
N_LINES = 8192
N_CORES = 8
TOK = N_LINES // N_CORES    # 1024 tokens per core
ROWS = TOK + HALO           # 1088 combined rows per core
GRP = 256                   # tokens per attention group
NG = TOK // GRP             # 4 groups per (head)
KEYS = GRP + HALO           # 320 keys per group
# key chunks (start, end) and their valid token windows (w0, w1)
CHUNKS = [(0, 128, 0, 128), (128, 256, 64, 256), (256, 320, 192, 256)]
VBLK = HEAD_DIM + 1         # 97: v head block + ones column
KC = EMBED_DIM // 128       # 6 contraction chunks of 128


def _build_program():
    import concourse.bacc as bacc
    import concourse.mybir as mybir
    from concourse.tile import TileContext

    f32 = mybir.dt.float32
    f32r = mybir.dt.float32r
    bf16 = mybir.dt.bfloat16
    D = EMBED_DIM

    nc = bacc.Bacc("TRN2", target_bir_lowering=False, debug=False,
                   enable_asserts=False, num_devices=N_CORES)

    xT = nc.dram_tensor("xT", [D, ROWS], f32r, kind="ExternalInput")
    wqT = nc.dram_tensor("wqT", [D, D], f32r, kind="ExternalInput")
    wkT = nc.dram_tensor("wkT", [D, D], f32r, kind="ExternalInput")
    wvT = nc.dram_tensor("wvT", [D, D], f32r, kind="ExternalInput")
    woT = nc.dram_tensor("woT", [D, D], f32r, kind="ExternalInput")
    bq = nc.dram_tensor("bq", [HEAD_DIM, NUM_HEADS], f32, kind="ExternalInput")
    bo2 = nc.dram_tensor("bo2", [128, KC], f32, kind="ExternalInput")
    m0 = nc.dram_tensor("m0", [128, 128], bf16, kind="ExternalInput")
    m1 = nc.dram_tensor("m1", [128, 192], bf16, kind="ExternalInput")
    m2 = nc.dram_tensor("m2", [64, 64], bf16, kind="ExternalInput")
    ones8 = nc.dram_tensor("ones8", [1, NUM_HEADS], bf16, kind="ExternalInput")
    out = nc.dram_tensor("out", [D, TOK], f32, kind="ExternalOutput")

    NVC = (ROWS + 127) // 128          # 9 v row-chunks (8x128 + 64)

    with TileContext(nc) as tc:
        with (
            tc.tile_pool(name="persist", bufs=1) as pers,
            tc.tile_pool(name="wpool", bufs=2) as wpool,
            tc.tile_pool(name="stage", bufs=3) as stage,
        ):
            qTh = pers.tile([HEAD_DIM, NUM_HEADS * TOK], f32r)
            kTh = pers.tile([HEAD_DIM, NUM_HEADS * ROWS], f32r)
            vt = pers.tile([128, NVC * NUM_HEADS * VBLK], bf16)
            ctxTh = pers.tile([HEAD_DIM, NUM_HEADS * TOK], f32r)
            bqt = pers.tile([HEAD_DIM, NUM_HEADS], f32)
            bo2t = pers.tile([128, KC], f32)
            mk0 = pers.tile([128, 128], bf16)
            mk1 = pers.tile([128, 192], bf16)
            mk2 = pers.tile([64, 64], bf16)
            nc.sync.dma_start(bqt[:], bq.ap())
            nc.sync.dma_start(bo2t[:], bo2.ap())
            nc.sync.dma_start(mk0[:], m0.ap())
            nc.sync.dma_start(mk1[:], m1.ap())
            nc.sync.dma_start(mk2[:], m2.ap())
            # ones columns of each v head-block (col 96 of every 97-block)
            for r in range(NVC):
                rows = min(128, ROWS - 128 * r)
                dst = vt[0:rows, r * NUM_HEADS * VBLK:(r + 1) * NUM_HEADS * VBLK]
                dst = dst.rearrange("p (b c) -> p b c", c=VBLK)[:, :, HEAD_DIM:VBLK]
                nc.sync.dma_start(dst, ones8.ap()[:, :, None].to_broadcast([rows, NUM_HEADS, 1]))

            with tc.tile_pool(name="xpool", bufs=1) as xpool, \
                 tc.tile_pool(name="ppsum", bufs=3, space="PSUM") as ppsum, \
                 tc.tile_pool(name="vpsum", bufs=2, space="PSUM") as vpsum:
                xt = xpool.tile([128, KC * ROWS], f32r)
                for c in range(KC):
                    nc.sync.dma_start(xt[:, c * ROWS:(c + 1) * ROWS],
                                      xT.ap()[c * 128:(c + 1) * 128, :])

                # ---- q / k projections (weight-stationary, head-aligned M=96)
                for name, wdram, dest, ncols, coff in (
                    ("q", wqT, qTh, TOK, OVERLAP),
                    ("k", wkT, kTh, ROWS, 0),
                ):
                    wt = wpool.tile([128, KC * D], f32r, tag="w", name=f"w_{name}")
                    for c in range(KC):
                        nc.sync.dma_start(wt[:, c * D:(c + 1) * D],
                                          wdram.ap()[c * 128:(c + 1) * 128, :])
                    nsz = [512] * (ncols // 512) + ([ncols % 512] if ncols % 512 else [])
                    for h in range(NUM_HEADS):
                        n0 = 0
                        for sz in nsz:
                            ps = ppsum.tile([HEAD_DIM, 512], f32, tag="pqk", name="ps_qk")
                            for c in range(KC):
                                nc.tensor.matmul(
                                    ps[:, 0:sz],
                                    wt[:, c * D + h * HEAD_DIM: c * D + (h + 1) * HEAD_DIM],
                                    xt[:, c * ROWS + coff + n0: c * ROWS + coff + n0 + sz],
                                    start=(c == 0), stop=(c == KC - 1))
                            if name == "q":
                                nc.vector.tensor_scalar_add(
                                    dest[:, h * ncols + n0: h * ncols + n0 + sz],
                                    ps[:, 0:sz], bqt[:, h:h + 1])
                            else:
                                nc.vector.tensor_copy(
                                    dest[:, h * ncols + n0: h * ncols + n0 + sz],
                                    ps[:, 0:sz])
                            n0 += sz

                # ---- v projection (x-stationary, natural layout, bf16 out)
                wv = wpool.tile([128, KC * D], f32r, tag="w", name="w_v")
                for c in range(KC):
                    nc.sync.dma_start(wv[:, c * D:(c + 1) * D],
                                      wvT.ap()[c * 128:(c + 1) * 128, :])
                for r in range(NVC):
                    rows = min(128, ROWS - 128 * r)
                    vp = vpsum.tile([128, D], f32, tag="pv", name="ps_v")
                    for nn in (0, 512):
                        sz = min(512, D - nn)
                        for c in range(KC):
                            nc.tensor.matmul(
                                vp[0:rows, nn:nn + sz],
                                xt[:, c * ROWS + 128 * r: c * ROWS + 128 * r + rows],
                                wv[:, c * D + nn: c * D + nn + sz],
                                start=(c == 0), stop=(c == KC - 1))
                    dst = vt[0:rows, r * NUM_HEADS * VBLK:(r + 1) * NUM_HEADS * VBLK]
                    dst = dst.rearrange("p (b c) -> p b c", c=VBLK)[:, :, 0:HEAD_DIM]
                    src = vp[0:rows, :].rearrange("p (b c) -> p b c", c=HEAD_DIM)
                    nc.scalar.copy(dst, src)

            # ---- attention groups
            with tc.tile_pool(name="apool", bufs=2) as apool, \
                 tc.tile_pool(name="apsum", bufs=2, space="PSUM") as apsum:
                masks = [mk0, mk1, mk2]
                for h in range(NUM_HEADS):
                    for g in range(NG):
                        ctx_ps = apsum.tile([VBLK, GRP], f32, tag="ctx", name="ctx_ps")
                        for c, (k0, k1, w0, w1) in enumerate(CHUNKS):
                            ksz = k1 - k0
                            s_ps = apsum.tile([128, GRP], f32, tag=f"s{c}", name=f"s_ps{c}")
                            nc.tensor.matmul(
                                s_ps[0:ksz, :],
                                kTh[:, h * ROWS + GRP * g + k0: h * ROWS + GRP * g + k1],
                                qTh[:, h * TOK + GRP * g: h * TOK + GRP * (g + 1)],
                                start=True, stop=True)
                            ex = apool.tile([128, GRP], bf16, tag=f"e{c}", name=f"ex{c}")
                            nc.scalar.activation(
                                ex[0:ksz, w0:w1], s_ps[0:ksz, w0:w1],
                                mybir.ActivationFunctionType.Exp)
                            nc.vector.tensor_tensor(
                                out=ex[0:ksz, w0:w1], in0=ex[0:ksz, w0:w1],
                                in1=masks[c][0:ksz, :], op=mybir.AluOpType.mult)
                            vchunk = 2 * g + c
                            nc.tensor.matmul(
                                ctx_ps[:, w0:w1],
                                vt[0:ksz, vchunk * NUM_HEADS * VBLK + h * VBLK:
                                   vchunk * NUM_HEADS * VBLK + (h + 1) * VBLK],
                                ex[0:ksz, w0:w1],
                                start=(c == 0), stop=(c == 2), skip_group_check=True)
                        rd = apool.tile([1, GRP], f32, tag="rd", name="rd")
                        nc.vector.reciprocal(rd[:], ctx_ps[HEAD_DIM:VBLK, :])
                        rdb = apool.tile([HEAD_DIM, GRP], f32, tag="rdb", name="rdb")
                        nc.gpsimd.partition_broadcast(rdb[:], rd[:])
                        nc.vector.tensor_tensor(
                            out=ctxTh[:, h * TOK + GRP * g: h * TOK + GRP * (g + 1)],
                            in0=ctx_ps[0:HEAD_DIM, :], in1=rdb[:],
                            op=mybir.AluOpType.mult)

            # ---- out projection (per-head contraction K=96)
            with tc.tile_pool(name="opool", bufs=2) as opool, \
                 tc.tile_pool(name="opsum", bufs=2, space="PSUM") as opsum:
                wo = opool.tile([HEAD_DIM, NUM_HEADS * D], f32r, tag="wo", bufs=1)
                for h in range(NUM_HEADS):
                    nc.sync.dma_start(wo[:, h * D:(h + 1) * D],
                                      woT.ap()[h * HEAD_DIM:(h + 1) * HEAD_DIM, :])
                for dc in range(KC):
                    for n0 in (0, 512):
                        op = opsum.tile([128, 512], f32, tag="po", name="ps_o")
                        for h in range(NUM_HEADS):
                            nc.tensor.matmul(
                                op[:],
                                wo[:, h * D + dc * 128: h * D + dc * 128 + 128],
                                ctxTh[:, h * TOK + n0: h * TOK + n0 + 512],
                                start=(h == 0), stop=(h == NUM_HEADS - 1))
                        ost = opool.tile([128, 512], f32, tag="ost", name="ost")
                        nc.vector.tensor_scalar_add(ost[:], op[:], bo2t[:, dc:dc + 1])
                        nc.sync.dma_start(out.ap()[dc * 128:(dc + 1) * 128, n0:n0 + 512],
                                          ost[:])
    nc.compile()
    return nc


_program_cache = {}


def _get_program():
    if "nc" not in _program_cache:
        _program_cache["nc"] = _build_program()
    return _program_cache["nc"]


def _host_masks():
    import ml_dtypes
    masks = []
    for (k0, k1, w0, w1) in CHUNKS:
        kk, mm = np.meshgrid(np.arange(k0, k1), np.arange(w0, w1), indexing="ij")
        d = kk - mm
        valid = (d >= 0) & (d <= HALO) & (d % 4 == 0) & (d != OVERLAP)
        masks.append(valid.astype(ml_dtypes.bfloat16))
    return masks


def kernel(main, begin, end, in_proj_w, in_proj_b, out_proj_w, out_proj_b):
    import ml_dtypes
    from concourse.bass_utils import run_bass_kernel_spmd

    main = np.asarray(main, np.float32)
    begin = np.asarray(begin, np.float32)
    end = np.asarray(end, np.float32)
    in_proj_w = np.asarray(in_proj_w, np.float32)
    in_proj_b = np.asarray(in_proj_b, np.float32)
    out_proj_w = np.asarray(out_proj_w, np.float32)
    out_proj_b = np.asarray(out_proj_b, np.float32)

    D = EMBED_DIM
    scale = HEAD_DIM ** -0.5
    wq, wk, wv = in_proj_w[:D], in_proj_w[D:2 * D], in_proj_w[2 * D:]
    bq_, bv = in_proj_b[:D], in_proj_b[2 * D:3 * D]
    combined = np.concatenate([begin, main, end], axis=0)  # [N + 64, D]

    wqT = np.ascontiguousarray(wq.T * scale)
    wkT = np.ascontiguousarray(wk.T)
    wvT = np.ascontiguousarray(wv.T)
    woT = np.ascontiguousarray(out_proj_w.T)
    bq_heads = np.ascontiguousarray((bq_ * scale).reshape(NUM_HEADS, HEAD_DIM).T)
    bo2 = out_proj_w @ bv + out_proj_b                      # [768]
    bo2_chunks = np.ascontiguousarray(bo2.reshape(KC, 128).T)
    masks = _host_masks()
    ones8 = np.ones((1, NUM_HEADS), ml_dtypes.bfloat16)

    shared = {
        "wqT": wqT, "wkT": wkT, "wvT": wvT, "woT": woT,
        "bq": bq_heads, "bo2": bo2_chunks,
        "m0": masks[0], "m1": masks[1], "m2": masks[2], "ones8": ones8,
    }
    in_maps = []
    for c in range(N_CORES):
        xT = np.ascontiguousarray(combined[c * TOK: c * TOK + ROWS].T)
        in_maps.append({**shared, "xT": xT})

    nc = _get_program()
    res = run_bass_kernel_spmd(nc, in_maps, core_ids=list(range(N_CORES)),
                               **_program_cache.get("run_kwargs", {}))
    _program_cache["last_result"] = res

    outp = np.empty((N_LINES, 2 * D), np.float32)
    outp[:, :D] = main
    for c in range(N_CORES):
        outp[c * TOK:(c + 1) * TOK, D:] = res.results[c]["out"].T
    return outp


# revision 3
# speedup vs baseline: 1.1573x; 1.1573x over previous
"""Trainium2 Bass kernel for dilated sliding-window attention (AttnWrapper).

Reference computation (all fp32):
  combined = [begin | main | end]                       # [8256, 768]
  keys[t]  = combined[t + 32 + off], off in +-{4..32}   # 16 dilated window keys
  q = (main @ wq.T + bq) * 96**-0.5
  k/v = keys @ w{k,v}.T + b{k,v}
  attn = softmax(q.k), ctx = attn.v, out = [main | ctx @ wo.T + bo]

Sharding: tokens across 8 cores (1024 each) with a 64-row halo of the
combined buffer; weights replicated. Each core computes attn_outT
[768, 1024]; the host transposes and concatenates with main.

Device-side math notes:
 - bk dropped (softmax shift invariance); bv folded into bo' = wo@bv + bo.
 - q scale and bq folded into wq/bq on the host.
 - Matmuls run as float32r (TF32-like, ~1.5e-4 rel err, 4x faster than fp32).
 - Attention probabilities and V are bf16 (A,V quantization ~2e-3 on ctx).
 - Scores are computed transposed (S[key, token]) in groups of 256 tokens x
   320 keys split into key-chunks of (128,128,64); only the valid token
   window of each chunk (128/192/64 wide) is exp'd/masked/matmul'd.
 - A ones-column appended to each V head block makes the ctx matmul also
   produce the softmax denominator (row 96 of the ctx PSUM tile).
"""

import numpy as np

EMBED_DIM = 768
NUM_HEADS = 8
HEAD_DIM = 96
OVERLAP = 32
HALO = 2 * OVERLAP          # 64 extra combined rows per core
N_LINES = 8192
N_CORES = 8
TOK = N_LINES // N_CORES    # 1024 tokens per core
ROWS = TOK + HALO           # 1088 combined rows per core
GRP = 256                   # tokens per attention group
NG = TOK // GRP             # 4 groups per (head)
KEYS = GRP + HALO           # 320 keys per group
# key chunks (start, end) and their valid token windows (w0, w1)
CHUNKS = [(0, 128, 0, 128), (128, 256, 64, 256), (256, 320, 192, 256)]
VBLK = HEAD_DIM + 1         # 97: v head block + ones column
KC = EMBED_DIM // 128       # 6 contraction chunks of 128


def _build_program():
    import concourse.bacc as bacc
    import concourse.mybir as mybir
    from concourse.tile import TileContext

    f32 = mybir.dt.float32
    f32r = mybir.dt.float32r
    bf16 = mybir.dt.bfloat16
    D = EMBED_DIM

    nc = bacc.Bacc("TRN2", target_bir_lowering=False, debug=False,
                   enable_asserts=False, num_devices=N_CORES)

    xT = nc.dram_tensor("xT", [D, ROWS], f32r, kind="ExternalInput")
    wqT = nc.dram_tensor("wqT", [D, D], f32r, kind="ExternalInput")
    wkT = nc.dram_tensor("wkT", [D, D], f32r, kind="ExternalInput")
    wvT = nc.dram_tensor("wvT", [D, D], f32r, kind="ExternalInput")
    woT = nc.dram_tensor("woT", [D, D], f32r, kind="ExternalInput")
    bq = nc.dram_tensor("bq", [HEAD_DIM, NUM_HEADS], f32, kind="ExternalInput")
    bo2 = nc.dram_tensor("bo2", [128, KC], f32, kind="ExternalInput")
    m0 = nc.dram_tensor("m0", [128, 128], bf16, kind="ExternalInput")
    m1 = nc.dram_tensor("m1", [128, 192], bf16, kind="ExternalInput")
    m2 = nc.dram_tensor("m2", [64, 64], bf16, kind="ExternalInput")
    ones8 = nc.dram_tensor("ones8", [1, NUM_HEADS], bf16, kind="ExternalInput")
    out = nc.dram_tensor("out", [D, TOK], f32, kind="ExternalOutput")

    NVC = (ROWS + 127) // 128          # 9 v row-chunks (8x128 + 64)

    with TileContext(nc) as tc:
        with (
            tc.tile_pool(name="persist", bufs=1) as pers,
            tc.tile_pool(name="wpool", bufs=2) as wpool,
            tc.tile_pool(name="stage", bufs=3) as stage,
        ):
            qTh = pers.tile([HEAD_DIM, NUM_HEADS * TOK], f32r)
            kTh = pers.tile([HEAD_DIM, NUM_HEADS * ROWS], f32r)
            vt = pers.tile([128, NVC * NUM_HEADS * VBLK], bf16)
            ctxTh = pers.tile([HEAD_DIM, NUM_HEADS * TOK], f32r)
            bqt = pers.tile([HEAD_DIM, NUM_HEADS], f32)
            bo2t = pers.tile([128, KC], f32)
            mk0 = pers.tile([128, 128], bf16)
            mk1 = pers.tile([128, 192], bf16)
            mk2 = pers.tile([64, 64], bf16)
            nc.sync.dma_start(bqt[:], bq.ap())
            nc.sync.dma_start(bo2t[:], bo2.ap())
            nc.sync.dma_start(mk0[:], m0.ap())
            nc.sync.dma_start(mk1[:], m1.ap())
            nc.sync.dma_start(mk2[:], m2.ap())
            # ones columns of each v head-block (col 96 of every 97-block)
            for r in range(NVC):
                rows = min(128, ROWS - 128 * r)
                dst = vt[0:rows, r * NUM_HEADS * VBLK:(r + 1) * NUM_HEADS * VBLK]
                dst = dst.rearrange("p (b c) -> p b c", c=VBLK)[:, :, HEAD_DIM:VBLK]
                nc.gpsimd.memset(dst, 1.0)

            with tc.tile_pool(name="xpool", bufs=1) as xpool, \
                 tc.tile_pool(name="ppsum", bufs=3, space="PSUM") as ppsum, \
                 tc.tile_pool(name="vpsum", bufs=2, space="PSUM") as vpsum:
                xt = xpool.tile([128, KC * ROWS], f32r)
                for c in range(KC):
                    nc.sync.dma_start(xt[:, c * ROWS:(c + 1) * ROWS],
                                      xT.ap()[c * 128:(c + 1) * 128, :])

                # ---- q / k projections (weight-stationary, head-aligned M=96)
                for name, wdram, dest, ncols, coff in (
                    ("q", wqT, qTh, TOK, OVERLAP),
                    ("k", wkT, kTh, ROWS, 0),
                ):
                    wt = wpool.tile([128, KC * D], f32r, tag="w", name=f"w_{name}")
                    for c in range(KC):
                        nc.sync.dma_start(wt[:, c * D:(c + 1) * D],
                                          wdram.ap()[c * 128:(c + 1) * 128, :])
                    nsz = [512] * (ncols // 512) + ([ncols % 512] if ncols % 512 else [])
                    for h in range(NUM_HEADS):
                        n0 = 0
                        for sz in nsz:
                            ps = ppsum.tile([HEAD_DIM, 512], f32, tag="pqk", name="ps_qk")
                            for c in range(KC):
                                nc.tensor.matmul(
                                    ps[:, 0:sz],
                                    wt[:, c * D + h * HEAD_DIM: c * D + (h + 1) * HEAD_DIM],
                                    xt[:, c * ROWS + coff + n0: c * ROWS + coff + n0 + sz],
                                    start=(c == 0), stop=(c == KC - 1))
                            if name == "q":
                                nc.vector.tensor_scalar_add(
                                    dest[:, h * ncols + n0: h * ncols + n0 + sz],
                                    ps[:, 0:sz], bqt[:, h:h + 1])
                            else:
                                nc.vector.tensor_copy(
                                    dest[:, h * ncols + n0: h * ncols + n0 + sz],
                                    ps[:, 0:sz])
                            n0 += sz

                # ---- v projection (x-stationary, natural layout, bf16 out)
                wv = wpool.tile([128, KC * D], f32r, tag="w", name="w_v")
                for c in range(KC):
                    nc.sync.dma_start(wv[:, c * D:(c + 1) * D],
                                      wvT.ap()[c * 128:(c + 1) * 128, :])
                for r in range(NVC):
                    rows = min(128, ROWS - 128 * r)
                    vp = vpsum.tile([128, D], f32, tag="pv", name="ps_v")
                    for nn in (0, 512):
                        sz = min(512, D - nn)
                        for c in range(KC):
                            nc.tensor.matmul(
                                vp[0:rows, nn:nn + sz],
                                xt[:, c * ROWS + 128 * r: c * ROWS + 128 * r + rows],
                                wv[:, c * D + nn: c * D + nn + sz],
                                start=(c == 0), stop=(c == KC - 1))
                    dst = vt[0:rows, r * NUM_HEADS * VBLK:(r + 1) * NUM_HEADS * VBLK]
                    dst = dst.rearrange("p (b c) -> p b c", c=VBLK)[:, :, 0:HEAD_DIM]
                    src = vp[0:rows, :].rearrange("p (b c) -> p b c", c=HEAD_DIM)
                    nc.scalar.copy(dst, src)

            # ---- attention groups
            with tc.tile_pool(name="apool", bufs=2) as apool, \
                 tc.tile_pool(name="apsum", bufs=2, space="PSUM") as apsum:
                masks = [mk0, mk1, mk2]
                for h in range(NUM_HEADS):
                    for g in range(NG):
                        ctx_ps = apsum.tile([VBLK, GRP], f32, tag="ctx", name="ctx_ps")
                        for c, (k0, k1, w0, w1) in enumerate(CHUNKS):
                            ksz = k1 - k0
                            s_ps = apsum.tile([128, GRP], f32, tag=f"s{c}", name=f"s_ps{c}")
                            nc.tensor.matmul(
                                s_ps[0:ksz, :],
                                kTh[:, h * ROWS + GRP * g + k0: h * ROWS + GRP * g + k1],
                                qTh[:, h * TOK + GRP * g: h * TOK + GRP * (g + 1)],
                                start=True, stop=True)
                            ex = apool.tile([128, GRP], bf16, tag=f"e{c}", name=f"ex{c}")
                            nc.scalar.activation(
                                ex[0:ksz, w0:w1], s_ps[0:ksz, w0:w1],
                                mybir.ActivationFunctionType.Exp)
                            nc.vector.tensor_tensor(
                                out=ex[0:ksz, w0:w1], in0=ex[0:ksz, w0:w1],
                                in1=masks[c][0:ksz, :], op=mybir.AluOpType.mult)
                            vchunk = 2 * g + c
                            nc.tensor.matmul(
                                ctx_ps[:, w0:w1],
                                vt[0:ksz, vchunk * NUM_HEADS * VBLK + h * VBLK:
                                   vchunk * NUM_HEADS * VBLK + (h + 1) * VBLK],
                                ex[0:ksz, w0:w1],
                                start=(c == 0), stop=(c == 2), skip_group_check=True)
                        # 1/denom via exp(-ln(d)) on ACT: DVE reciprocal is ~7cyc/elem
                        # and runs on a single partition lane here (1.8us/group).
                        rl = apool.tile([1, GRP], f32, tag="rl", name="rl")
                        nc.scalar.activation(rl[:], ctx_ps[HEAD_DIM:VBLK, :],
                                             mybir.ActivationFunctionType.Ln)
                        rd = apool.tile([1, GRP], f32, tag="rd", name="rd")
                        nc.scalar.activation(rd[:], rl[:],
                                             mybir.ActivationFunctionType.Exp,
                                             scale=-1.0)
                        rdb = apool.tile([HEAD_DIM, GRP], f32, tag="rdb", name="rdb")
                        nc.gpsimd.partition_broadcast(rdb[:], rd[:])
                        nc.vector.tensor_tensor(
                            out=ctxTh[:, h * TOK + GRP * g: h * TOK + GRP * (g + 1)],
                            in0=ctx_ps[0:HEAD_DIM, :], in1=rdb[:],
                            op=mybir.AluOpType.mult)

            # ---- out projection (per-head contraction K=96)
            with tc.tile_pool(name="opool", bufs=2) as opool, \
                 tc.tile_pool(name="opsum", bufs=2, space="PSUM") as opsum:
                wo = opool.tile([HEAD_DIM, NUM_HEADS * D], f32r, tag="wo", bufs=1)
                for h in range(NUM_HEADS):
                    nc.sync.dma_start(wo[:, h * D:(h + 1) * D],
                                      woT.ap()[h * HEAD_DIM:(h + 1) * HEAD_DIM, :])
                for dc in range(KC):
                    for n0 in (0, 512):
                        op = opsum.tile([128, 512], f32, tag="po", name="ps_o")
                        for h in range(NUM_HEADS):
                            nc.tensor.matmul(
                                op[:],
                                wo[:, h * D + dc * 128: h * D + dc * 128 + 128],
                                ctxTh[:, h * TOK + n0: h * TOK + n0 + 512],
                                start=(h == 0), stop=(h == NUM_HEADS - 1))
                        ost = opool.tile([128, 512], f32, tag="ost", name="ost")
                        nc.vector.tensor_scalar_add(ost[:], op[:], bo2t[:, dc:dc + 1])
                        nc.sync.dma_start(out.ap()[dc * 128:(dc + 1) * 128, n0:n0 + 512],
                                          ost[:])
    nc.compile()
    return nc


_program_cache = {}


def _get_program():
    if "nc" not in _program_cache:
        _program_cache["nc"] = _build_program()
    return _program_cache["nc"]


def _host_masks():
    import ml_dtypes
    masks = []
    for (k0, k1, w0, w1) in CHUNKS:
        kk, mm = np.meshgrid(np.arange(k0, k1), np.arange(w0, w1), indexing="ij")
        d = kk - mm
        valid = (d >= 0) & (d <= HALO) & (d % 4 == 0) & (d != OVERLAP)
        masks.append(valid.astype(ml_dtypes.bfloat16))
    return masks


def kernel(main, begin, end, in_proj_w, in_proj_b, out_proj_w, out_proj_b):
    import ml_dtypes
    from concourse.bass_utils import run_bass_kernel_spmd

    main = np.asarray(main, np.float32)
    begin = np.asarray(begin, np.float32)
    end = np.asarray(end, np.float32)
    in_proj_w = np.asarray(in_proj_w, np.float32)
    in_proj_b = np.asarray(in_proj_b, np.float32)
    out_proj_w = np.asarray(out_proj_w, np.float32)
    out_proj_b = np.asarray(out_proj_b, np.float32)

    D = EMBED_DIM
    scale = HEAD_DIM ** -0.5
    wq, wk, wv = in_proj_w[:D], in_proj_w[D:2 * D], in_proj_w[2 * D:]
    bq_, bv = in_proj_b[:D], in_proj_b[2 * D:3 * D]
    combined = np.concatenate([begin, main, end], axis=0)  # [N + 64, D]

    wqT = np.ascontiguousarray(wq.T * scale)
    wkT = np.ascontiguousarray(wk.T)
    wvT = np.ascontiguousarray(wv.T)
    woT = np.ascontiguousarray(out_proj_w.T)
    bq_heads = np.ascontiguousarray((bq_ * scale).reshape(NUM_HEADS, HEAD_DIM).T)
    bo2 = out_proj_w @ bv + out_proj_b                      # [768]
    bo2_chunks = np.ascontiguousarray(bo2.reshape(KC, 128).T)
    masks = _host_masks()
    ones8 = np.ones((1, NUM_HEADS), ml_dtypes.bfloat16)

    shared = {
        "wqT": wqT, "wkT": wkT, "wvT": wvT, "woT": woT,
        "bq": bq_heads, "bo2": bo2_chunks,
        "m0": masks[0], "m1": masks[1], "m2": masks[2], "ones8": ones8,
    }
    in_maps = []
    for c in range(N_CORES):
        xT = np.ascontiguousarray(combined[c * TOK: c * TOK + ROWS].T)
        in_maps.append({**shared, "xT": xT})

    nc = _get_program()
    res = run_bass_kernel_spmd(nc, in_maps, core_ids=list(range(N_CORES)),
                               **_program_cache.get("run_kwargs", {}))
    _program_cache["last_result"] = res

    outp = np.empty((N_LINES, 2 * D), np.float32)
    outp[:, :D] = main
    for c in range(N_CORES):
        outp[c * TOK:(c + 1) * TOK, D:] = res.results[c]["out"].T
    return outp


# revision 16
# speedup vs baseline: 1.5589x; 1.3471x over previous
"""Trainium2 Bass kernel for dilated sliding-window attention (AttnWrapper).

Reference computation (all fp32):
  combined = [begin | main | end]                       # [8256, 768]
  keys[t]  = combined[t + 32 + off], off in +-{4..32}   # 16 dilated window keys
  q = (main @ wq.T + bq) * 96**-0.5
  k/v = keys @ w{k,v}.T + b{k,v}
  attn = softmax(q.k), ctx = attn.v, out = [main | ctx @ wo.T + bo]

Sharding: tokens across 8 cores (1024 each) with a 64-row halo of the
combined buffer; weights replicated. Each core computes attn_outT
[768, 1024]; the host transposes and concatenates with main.

Device-side math notes:
 - bk dropped (softmax shift invariance); bv folded into bo' = wo@bv + bo.
 - q scale and bq folded into wq/bq on the host.
 - Matmuls run as float32r (TF32-like, ~1.5e-4 rel err, 4x faster than fp32).
 - Attention probabilities and V are bf16 (A,V quantization ~2e-3 on ctx).
 - Scores are computed transposed (S[key, token]) in groups of 256 tokens x
   320 keys split into key-chunks of (128,128,64); only the valid token
   window of each chunk (128/192/64 wide) is exp'd/masked/matmul'd.
 - A ones-column appended to each V head block makes the ctx matmul also
   produce the softmax denominator (row 96 of the ctx PSUM tile).
 - q/k/v tensors are split per-head / per-row-chunk so the Tile scheduler can
   overlap the attention pipeline with the tail of the projections.
"""

import numpy as np

EMBED_DIM = 768
NUM_HEADS = 8
HEAD_DIM = 96
OVERLAP = 32
HALO = 2 * OVERLAP          # 64 extra combined rows per core
N_LINES = 8192
N_CORES = 8
TOK = N_LINES // N_CORES    # 1024 tokens per core
ROWS = TOK + HALO           # 1088 combined rows per core
GRP = 512                   # tokens per attention group
NG = TOK // GRP             # 2 groups
# key chunks (start, end), valid token windows (w0, w1), mask index
CHUNKS = [(0, 128, 0, 128, 0), (128, 256, 64, 256, 1), (256, 384, 192, 384, 1),
          (384, 512, 320, 512, 1), (512, 576, 448, 512, 2)]
VBLK = HEAD_DIM + 1         # 97: v head block + ones column
KC = EMBED_DIM // 128       # 6 contraction chunks of 128
NVC = (ROWS + 127) // 128   # 9 v row-chunks (8x128 + 64)


def _build_program():
    import concourse.bacc as bacc
    import concourse.mybir as mybir
    from concourse.tile import TileContext

    f32 = mybir.dt.float32
    f32r = mybir.dt.float32r
    bf16 = mybir.dt.bfloat16
    D = EMBED_DIM

    nc = bacc.Bacc("TRN2", target_bir_lowering=False, debug=False,
                   enable_asserts=False, num_devices=N_CORES)

    xT = nc.dram_tensor("xT", [D, ROWS], f32r, kind="ExternalInput")
    wqT = nc.dram_tensor("wqT", [D, D], f32r, kind="ExternalInput")
    wkT = nc.dram_tensor("wkT", [D, D], f32r, kind="ExternalInput")
    wvT = nc.dram_tensor("wvT", [D, D], bf16, kind="ExternalInput")
    woT = nc.dram_tensor("woT", [D, D], f32r, kind="ExternalInput")
    bq = nc.dram_tensor("bq", [HEAD_DIM, NUM_HEADS], f32, kind="ExternalInput")
    bo2 = nc.dram_tensor("bo2", [128, KC], f32, kind="ExternalInput")
    m0 = nc.dram_tensor("m0", [128, 128], bf16, kind="ExternalInput")
    m1 = nc.dram_tensor("m1", [128, 192], bf16, kind="ExternalInput")
    m2 = nc.dram_tensor("m2", [64, 64], bf16, kind="ExternalInput")
    out = nc.dram_tensor("out", [D, TOK], f32, kind="ExternalOutput")

    with TileContext(nc) as tc:
        with tc.tile_pool(name="persist", bufs=1) as pers:
            qTh = [pers.tile([HEAD_DIM, TOK], f32r, name=f"qTh{h}")
                   for h in range(NUM_HEADS)]
            kTh = [pers.tile([HEAD_DIM, ROWS], f32r, name=f"kTh{h}")
                   for h in range(NUM_HEADS)]
            vt = [pers.tile([128, NUM_HEADS * VBLK], bf16, name=f"vt{r}")
                  for r in range(NVC)]
            bqt = pers.tile([HEAD_DIM, NUM_HEADS], f32)
            bo2t = pers.tile([128, KC], f32)
            mk0 = pers.tile([128, 128], bf16)
            mk1 = pers.tile([128, 192], bf16)
            mk2 = pers.tile([64, 64], bf16)
            masks = [mk0, mk1, mk2]
            nc.sync.dma_start(bqt[:], bq.ap())
            nc.sync.dma_start(bo2t[:], bo2.ap())
            nc.sync.dma_start(mk0[:], m0.ap())
            nc.sync.dma_start(mk1[:], m1.ap())
            nc.sync.dma_start(mk2[:], m2.ap())
            for r in range(NVC):
                rows = min(128, ROWS - 128 * r)
                dst = vt[r][0:rows, :].rearrange("p (b c) -> p b c", c=VBLK)
                nc.gpsimd.memset(dst[:, :, HEAD_DIM:VBLK], 1.0)

            with tc.tile_pool(name="xpool", bufs=1) as xpool, \
                 tc.tile_pool(name="wpool", bufs=2) as wpool:

                xt = xpool.tile([128, KC * ROWS], f32r)
                xtb = xpool.tile([128, KC * ROWS], bf16)
                for c in range(KC):
                    nc.sync.dma_start(xt[:, c * ROWS:(c + 1) * ROWS],
                                      xT.ap()[c * 128:(c + 1) * 128, :])
                    nc.vector.tensor_copy(xtb[:, c * ROWS:(c + 1) * ROWS],
                                          xt[:, c * ROWS:(c + 1) * ROWS].bitcast(f32))

                with tc.tile_pool(name="vpsum", bufs=2, space="PSUM") as vpsum, \
                     tc.tile_pool(name="ppsum", bufs=2, space="PSUM") as ppsum:
                    # ---- v projection (x-stationary, natural layout, bf16)
                    wv = wpool.tile([128, KC * D], bf16, tag="w", name="w_v")
                    for c in range(KC):
                        nc.sync.dma_start(wv[:, c * D:(c + 1) * D],
                                          wvT.ap()[c * 128:(c + 1) * 128, :])
                    for r in range(NVC):
                        rows = min(128, ROWS - 128 * r)
                        pv0 = vpsum.tile([128, 512], f32, tag="pv0", name="pv0")
                        pv1 = vpsum.tile([128, 256], f32, tag="pv1", name="pv1")
                        vps = [pv0, pv1]
                        for c in range(KC):
                            for i, (nn, sz) in enumerate(((0, 512), (512, 256))):
                                nc.tensor.matmul(
                                    vps[i][0:rows, 0:sz],
                                    xtb[:, c * ROWS + 128 * r: c * ROWS + 128 * r + rows],
                                    wv[:, c * D + nn: c * D + nn + sz],
                                    start=(c == 0), stop=(c == KC - 1))
                        dst = vt[r][0:rows, :].rearrange("p (b c) -> p b c", c=VBLK)
                        nc.scalar.copy(
                            dst[:, 0:5, 0:HEAD_DIM],
                            pv0[0:rows, 0:5 * HEAD_DIM]
                            .rearrange("p (b c) -> p b c", c=HEAD_DIM))
                        # head 5 straddles the 512 boundary: cols 480:512 | 0:64
                        nc.scalar.copy(dst[:, 5, 0:32], pv0[0:rows, 480:512])
                        nc.scalar.copy(dst[:, 5, 32:HEAD_DIM], pv1[0:rows, 0:64])
                        nc.scalar.copy(
                            dst[:, 6:8, 0:HEAD_DIM],
                            pv1[0:rows, 64:64 + 2 * HEAD_DIM]
                            .rearrange("p (b c) -> p b c", c=HEAD_DIM))

                    # ---- q / k projections (weight-stationary, M=96 per head)
                    for name, wdram, dest, ncols, coff in (
                        ("q", wqT, qTh, TOK, OVERLAP),
                        ("k", wkT, kTh, ROWS, 0),
                    ):
                        wt = wpool.tile([128, KC * D], f32r, tag="w", name=f"w_{name}")
                        for c in range(KC):
                            nc.sync.dma_start(wt[:, c * D:(c + 1) * D],
                                              wdram.ap()[c * 128:(c + 1) * 128, :])
                        nsz = [512] * (ncols // 512) + \
                              ([ncols % 512] if ncols % 512 else [])
                        for h in range(NUM_HEADS):
                            n0 = 0
                            for sz in nsz:
                                ps = ppsum.tile([HEAD_DIM, 512], f32, tag="pqk",
                                                name="ps_qk")
                                for c in range(KC):
                                    nc.tensor.matmul(
                                        ps[:, 0:sz],
                                        wt[:, c * D + h * HEAD_DIM:
                                           c * D + (h + 1) * HEAD_DIM],
                                        xt[:, c * ROWS + coff + n0:
                                           c * ROWS + coff + n0 + sz],
                                        start=(c == 0), stop=(c == KC - 1))
                                if name == "q":
                                    nc.vector.tensor_scalar_add(
                                        dest[h][:, n0:n0 + sz], ps[:, 0:sz],
                                        bqt[:, h:h + 1])
                                else:
                                    nc.vector.tensor_copy(
                                        dest[h][:, n0:n0 + sz], ps[:, 0:sz])
                                n0 += sz

            # ---- attention + normalization + out-projection, software-pipelined
            with tc.tile_pool(name="apool", bufs=2) as apool, \
                 tc.tile_pool(name="upool", bufs=1) as upool, \
                 tc.tile_pool(name="opool", bufs=2) as opool, \
                 tc.tile_pool(name="apsum", bufs=2, space="PSUM") as apsum, \
                 tc.tile_pool(name="opsum", bufs=2, space="PSUM") as opsum:
                wo = opool.tile([HEAD_DIM, NUM_HEADS * D], f32r, tag="wo", bufs=1)
                for h in range(NUM_HEADS):
                    nc.sync.dma_start(wo[:, h * D:(h + 1) * D],
                                      woT.ap()[h * HEAD_DIM:(h + 1) * HEAD_DIM, :])
                ctxU = [upool.tile([VBLK, NUM_HEADS * GRP], f32, name=f"ctxU{g}",
                                   tag=f"ctxU{g}") for g in range(NG)]
                ctxH = [upool.tile([HEAD_DIM, NUM_HEADS * GRP], f32r, name=f"ctxH{i}",
                                   tag=f"ctxH{i}") for i in range(NG)]

                def attention_group(g, h):
                    ctx_ps = apsum.tile([VBLK, GRP], f32, tag="ctx", name="ctx_ps")
                    for c, (k0, k1, w0, w1, mi) in enumerate(CHUNKS):
                        ksz = k1 - k0
                        win = w1 - w0
                        s_ps = apsum.tile([128, GRP], f32, tag="s", name="s_ps",
                                          bufs=4)
                        nc.tensor.matmul(
                            s_ps[0:ksz, :],
                            kTh[h][:, GRP * g + k0: GRP * g + k1],
                            qTh[h][:, GRP * g: GRP * (g + 1)],
                            start=True, stop=True)
                        ex = apool.tile([128, 192], bf16, tag="ex", name="ex",
                                        bufs=6)
                        nc.scalar.activation(
                            ex[0:ksz, 0:win], s_ps[0:ksz, w0:w1],
                            mybir.ActivationFunctionType.Exp)
                        nc.vector.tensor_tensor(
                            out=ex[0:ksz, 0:win], in0=ex[0:ksz, 0:win],
                            in1=masks[mi][0:ksz, :], op=mybir.AluOpType.mult)
                        nc.tensor.matmul(
                            ctx_ps[:, w0:w1],
                            vt[4 * g + c][0:ksz, h * VBLK:(h + 1) * VBLK],
                            ex[0:ksz, 0:win],
                            start=(c == 0), stop=(c == len(CHUNKS) - 1),
                            skip_group_check=True)
                    nc.vector.tensor_copy(
                        ctxU[g][:, h * GRP:(h + 1) * GRP], ctx_ps[:])

                def normalize_half(g):
                    # 1/denom = exp(-ln(d)) on ACT (DVE reciprocal is 7cyc/elem).
                    # Goes through a partition-0 tile: partition_broadcast
                    # replicates partition 0, not the AP's partition offset.
                    rl = apool.tile([1, NUM_HEADS * GRP], f32, tag="rl",
                                    name="rl", bufs=1)
                    nc.scalar.activation(rl[:], ctxU[g][HEAD_DIM:VBLK, :],
                                         mybir.ActivationFunctionType.Ln)
                    nc.scalar.activation(rl[:], rl[:],
                                         mybir.ActivationFunctionType.Exp, scale=-1.0)
                    rdb = apool.tile([HEAD_DIM, NUM_HEADS * GRP], f32,
                                     tag="rdb", name="rdb", bufs=1)
                    nc.gpsimd.partition_broadcast(rdb[:], rl[:])
                    nc.vector.tensor_tensor(
                        out=ctxH[g][:], in0=ctxU[g][0:HEAD_DIM, :], in1=rdb[:],
                        op=mybir.AluOpType.mult)

                def outproj_half(i):
                    for dc in range(KC):
                        op = opsum.tile([128, 512], f32, tag="po", name="ps_o")
                        for h in range(NUM_HEADS):
                            nc.tensor.matmul(
                                op[:],
                                wo[:, h * D + dc * 128: h * D + dc * 128 + 128],
                                ctxH[i][:, h * GRP:(h + 1) * GRP],
                                start=(h == 0), stop=(h == NUM_HEADS - 1))
                        ost = opool.tile([128, 512], f32, tag="ost", name="ost")
                        nc.vector.tensor_scalar_add(ost[:], op[:], bo2t[:, dc:dc + 1])
                        nc.sync.dma_start(
                            out.ap()[dc * 128:(dc + 1) * 128, i * 512:(i + 1) * 512],
                            ost[:])

                for g in range(NG):
                    for h in range(NUM_HEADS):
                        attention_group(g, h)
                    normalize_half(g)
                    outproj_half(g)
    nc.compile()
    return nc


_program_cache = {}


def _get_program():
    if "nc" not in _program_cache:
        _program_cache["nc"] = _build_program()
    return _program_cache["nc"]


def _host_masks():
    # Three mask patterns: d = key - token offset within the chunk window.
    # m0 (first chunk): d = kk - mm; m1/m2 (later chunks): d = kk - mm + 64.
    import ml_dtypes
    masks = []
    for (nk, nw, off) in ((128, 128, 0), (128, 192, HALO), (64, 64, HALO)):
        kk, mm = np.meshgrid(np.arange(nk), np.arange(nw), indexing="ij")
        d = kk - mm + off
        valid = (d >= 0) & (d <= HALO) & (d % 4 == 0) & (d != OVERLAP)
        masks.append(valid.astype(ml_dtypes.bfloat16))
    return masks


def kernel(main, begin, end, in_proj_w, in_proj_b, out_proj_w, out_proj_b):
    import ml_dtypes
    from concourse.bass_utils import run_bass_kernel_spmd

    main = np.asarray(main, np.float32)
    begin = np.asarray(begin, np.float32)
    end = np.asarray(end, np.float32)
    in_proj_w = np.asarray(in_proj_w, np.float32)
    in_proj_b = np.asarray(in_proj_b, np.float32)
    out_proj_w = np.asarray(out_proj_w, np.float32)
    out_proj_b = np.asarray(out_proj_b, np.float32)

    D = EMBED_DIM
    scale = HEAD_DIM ** -0.5
    wq, wk, wv = in_proj_w[:D], in_proj_w[D:2 * D], in_proj_w[2 * D:]
    bq_, bv = in_proj_b[:D], in_proj_b[2 * D:3 * D]
    combined = np.concatenate([begin, main, end], axis=0)  # [N + 64, D]

    wqT = np.ascontiguousarray(wq.T * scale)
    wkT = np.ascontiguousarray(wk.T)
    wvT = np.ascontiguousarray(wv.T).astype(ml_dtypes.bfloat16)
    woT = np.ascontiguousarray(out_proj_w.T)
    bq_heads = np.ascontiguousarray((bq_ * scale).reshape(NUM_HEADS, HEAD_DIM).T)
    bo2 = out_proj_w @ bv + out_proj_b                      # [768]
    bo2_chunks = np.ascontiguousarray(bo2.reshape(KC, 128).T)
    masks = _host_masks()

    shared = {
        "wqT": wqT, "wkT": wkT, "wvT": wvT, "woT": woT,
        "bq": bq_heads, "bo2": bo2_chunks,
        "m0": masks[0], "m1": masks[1], "m2": masks[2],
    }
    in_maps = []
    for c in range(N_CORES):
        xTc = np.ascontiguousarray(combined[c * TOK: c * TOK + ROWS].T)
        in_maps.append({**shared, "xT": xTc})

    nc = _get_program()
    res = run_bass_kernel_spmd(nc, in_maps, core_ids=list(range(N_CORES)),
                               **_program_cache.get("run_kwargs", {}))
    _program_cache["last_result"] = res

    outp = np.empty((N_LINES, 2 * D), np.float32)
    outp[:, :D] = main
    for c in range(N_CORES):
        outp[c * TOK:(c + 1) * TOK, D:] = res.results[c]["out"].T
    return outp


# revision 17
# speedup vs baseline: 1.6246x; 1.0421x over previous
"""Trainium2 Bass kernel for dilated sliding-window attention (AttnWrapper).

Reference computation (all fp32):
  combined = [begin | main | end]                       # [8256, 768]
  keys[t]  = combined[t + 32 + off], off in +-{4..32}   # 16 dilated window keys
  q = (main @ wq.T + bq) * 96**-0.5
  k/v = keys @ w{k,v}.T + b{k,v}
  attn = softmax(q.k), ctx = attn.v, out = [main | ctx @ wo.T + bo]

Sharding: tokens across 8 cores (1024 each) with a 64-row halo of the
combined buffer; weights replicated. Each core computes attn_outT
[768, 1024]; the host transposes and concatenates with main.

Device-side math notes:
 - bk dropped (softmax shift invariance); bv folded into bo' = wo@bv + bo.
 - q scale and bq folded into wq/bq on the host.
 - Matmuls run as float32r (TF32-like, ~1.5e-4 rel err, 4x faster than fp32).
 - Attention probabilities and V are bf16 (A,V quantization ~2e-3 on ctx).
 - Scores are computed transposed (S[key, token]) in groups of 256 tokens x
   320 keys split into key-chunks of (128,128,64); only the valid token
   window of each chunk (128/192/64 wide) is exp'd/masked/matmul'd.
 - A ones-column appended to each V head block makes the ctx matmul also
   produce the softmax denominator (row 96 of the ctx PSUM tile).
 - q/k/v tensors are split per-head / per-row-chunk so the Tile scheduler can
   overlap the attention pipeline with the tail of the projections.
"""

import numpy as np

EMBED_DIM = 768
NUM_HEADS = 8
HEAD_DIM = 96
OVERLAP = 32
HALO = 2 * OVERLAP          # 64 extra combined rows per core
N_LINES = 8192
N_CORES = 8
TOK = N_LINES // N_CORES    # 1024 tokens per core
ROWS = TOK + HALO           # 1088 combined rows per core
GRP = 512                   # tokens per attention group
NG = TOK // GRP             # 2 groups
# key chunks (start, end), valid token windows (w0, w1), mask index
CHUNKS = [(0, 128, 0, 128, 0), (128, 256, 64, 256, 1), (256, 384, 192, 384, 1),
          (384, 512, 320, 512, 1), (512, 576, 448, 512, 2)]
VBLK = HEAD_DIM + 1         # 97: v head block + ones column
KC = EMBED_DIM // 128       # 6 contraction chunks of 128
NVC = (ROWS + 127) // 128   # 9 v row-chunks (8x128 + 64)


def _build_program():
    import concourse.bacc as bacc
    import concourse.mybir as mybir
    from concourse.tile import TileContext

    f32 = mybir.dt.float32
    f32r = mybir.dt.float32r
    bf16 = mybir.dt.bfloat16
    D = EMBED_DIM

    nc = bacc.Bacc("TRN2", target_bir_lowering=False, debug=False,
                   enable_asserts=False, num_devices=N_CORES)

    xT = nc.dram_tensor("xT", [D, ROWS], f32r, kind="ExternalInput")
    wqT = nc.dram_tensor("wqT", [D, D], f32r, kind="ExternalInput")
    wkT = nc.dram_tensor("wkT", [D, D], f32r, kind="ExternalInput")
    wvT = nc.dram_tensor("wvT", [D, D], bf16, kind="ExternalInput")
    woT = nc.dram_tensor("woT", [D, D], f32r, kind="ExternalInput")
    bq = nc.dram_tensor("bq", [HEAD_DIM, NUM_HEADS], f32, kind="ExternalInput")
    bo2 = nc.dram_tensor("bo2", [128, KC], f32, kind="ExternalInput")
    m0 = nc.dram_tensor("m0", [128, 128], bf16, kind="ExternalInput")
    m1 = nc.dram_tensor("m1", [128, 192], bf16, kind="ExternalInput")
    m2 = nc.dram_tensor("m2", [64, 64], bf16, kind="ExternalInput")
    out = nc.dram_tensor("out", [D, TOK], f32, kind="ExternalOutput")

    with TileContext(nc) as tc:
        with tc.tile_pool(name="persist", bufs=1) as pers:
            qTh = [pers.tile([HEAD_DIM, TOK], f32r, name=f"qTh{h}")
                   for h in range(NUM_HEADS)]
            kTh = [pers.tile([HEAD_DIM, ROWS], f32r, name=f"kTh{h}")
                   for h in range(NUM_HEADS)]
            vt = [pers.tile([128, NUM_HEADS * VBLK], bf16, name=f"vt{r}")
                  for r in range(NVC)]
            bqt = pers.tile([HEAD_DIM, NUM_HEADS], f32)
            bo2t = pers.tile([128, KC], f32)
            mk0 = pers.tile([128, 128], bf16)
            mk1 = pers.tile([128, 192], bf16)
            mk2 = pers.tile([64, 64], bf16)
            masks = [mk0, mk1, mk2]
            nc.sync.dma_start(bqt[:], bq.ap())
            nc.sync.dma_start(bo2t[:], bo2.ap())
            nc.sync.dma_start(mk0[:], m0.ap())
            nc.sync.dma_start(mk1[:], m1.ap())
            nc.sync.dma_start(mk2[:], m2.ap())
            for r in range(NVC):
                rows = min(128, ROWS - 128 * r)
                dst = vt[r][0:rows, :].rearrange("p (b c) -> p b c", c=VBLK)
                nc.gpsimd.memset(dst[:, :, HEAD_DIM:VBLK], 1.0)

            with tc.tile_pool(name="xpool", bufs=1) as xpool, \
                 tc.tile_pool(name="wpool", bufs=2) as wpool:

                xt = xpool.tile([128, KC * ROWS], f32r)
                xtb = xpool.tile([128, KC * ROWS], bf16)
                for c in range(KC):
                    nc.sync.dma_start(xt[:, c * ROWS:(c + 1) * ROWS],
                                      xT.ap()[c * 128:(c + 1) * 128, :])
                    nc.vector.tensor_copy(xtb[:, c * ROWS:(c + 1) * ROWS],
                                          xt[:, c * ROWS:(c + 1) * ROWS].bitcast(f32))

                with tc.tile_pool(name="vpsum", bufs=2, space="PSUM") as vpsum, \
                     tc.tile_pool(name="ppsum", bufs=2, space="PSUM") as ppsum:
                    # ---- v projection (x-stationary, natural layout, bf16)
                    wv = wpool.tile([128, KC * D], bf16, tag="w", name="w_v")
                    for c in range(KC):
                        nc.sync.dma_start(wv[:, c * D:(c + 1) * D],
                                          wvT.ap()[c * 128:(c + 1) * 128, :])
                    for r in range(NVC):
                        rows = min(128, ROWS - 128 * r)
                        pv0 = vpsum.tile([128, 512], f32, tag="pv0", name="pv0")
                        pv1 = vpsum.tile([128, 256], f32, tag="pv1", name="pv1")
                        vps = [pv0, pv1]
                        for c in range(KC):
                            for i, (nn, sz) in enumerate(((0, 512), (512, 256))):
                                nc.tensor.matmul(
                                    vps[i][0:rows, 0:sz],
                                    xtb[:, c * ROWS + 128 * r: c * ROWS + 128 * r + rows],
                                    wv[:, c * D + nn: c * D + nn + sz],
                                    start=(c == 0), stop=(c == KC - 1))
                        dst = vt[r][0:rows, :].rearrange("p (b c) -> p b c", c=VBLK)
                        nc.scalar.copy(
                            dst[:, 0:5, 0:HEAD_DIM],
                            pv0[0:rows, 0:5 * HEAD_DIM]
                            .rearrange("p (b c) -> p b c", c=HEAD_DIM))
                        # head 5 straddles the 512 boundary: cols 480:512 | 0:64
                        nc.scalar.copy(dst[:, 5, 0:32], pv0[0:rows, 480:512])
                        nc.scalar.copy(dst[:, 5, 32:HEAD_DIM], pv1[0:rows, 0:64])
                        nc.scalar.copy(
                            dst[:, 6:8, 0:HEAD_DIM],
                            pv1[0:rows, 64:64 + 2 * HEAD_DIM]
                            .rearrange("p (b c) -> p b c", c=HEAD_DIM))

                    # ---- q / k projections (weight-stationary, M=96 per head)
                    for name, wdram, dest, ncols, coff in (
                        ("q", wqT, qTh, TOK, OVERLAP),
                        ("k", wkT, kTh, ROWS, 0),
                    ):
                        wt = wpool.tile([128, KC * D], f32r, tag="w", name=f"w_{name}")
                        for c in range(KC):
                            nc.sync.dma_start(wt[:, c * D:(c + 1) * D],
                                              wdram.ap()[c * 128:(c + 1) * 128, :])
                        nsz = [512] * (ncols // 512) + \
                              ([ncols % 512] if ncols % 512 else [])
                        for h in range(NUM_HEADS):
                            n0 = 0
                            for sz in nsz:
                                ps = ppsum.tile([HEAD_DIM, 512], f32, tag="pqk",
                                                name="ps_qk")
                                for c in range(KC):
                                    nc.tensor.matmul(
                                        ps[:, 0:sz],
                                        wt[:, c * D + h * HEAD_DIM:
                                           c * D + (h + 1) * HEAD_DIM],
                                        xt[:, c * ROWS + coff + n0:
                                           c * ROWS + coff + n0 + sz],
                                        start=(c == 0), stop=(c == KC - 1))
                                if name == "q":
                                    nc.vector.tensor_scalar_add(
                                        dest[h][:, n0:n0 + sz], ps[:, 0:sz],
                                        bqt[:, h:h + 1])
                                else:
                                    nc.vector.tensor_copy(
                                        dest[h][:, n0:n0 + sz], ps[:, 0:sz])
                                n0 += sz

            # ---- attention + normalization + out-projection, software-pipelined
            with tc.tile_pool(name="apool", bufs=2) as apool, \
                 tc.tile_pool(name="upool", bufs=1) as upool, \
                 tc.tile_pool(name="opool", bufs=2) as opool, \
                 tc.tile_pool(name="apsum", bufs=2, space="PSUM") as apsum, \
                 tc.tile_pool(name="opsum", bufs=2, space="PSUM") as opsum:
                wo = opool.tile([HEAD_DIM, NUM_HEADS * D], f32r, tag="wo", bufs=1)
                for h in range(NUM_HEADS):
                    nc.sync.dma_start(wo[:, h * D:(h + 1) * D],
                                      woT.ap()[h * HEAD_DIM:(h + 1) * HEAD_DIM, :])
                ctxU = [upool.tile([VBLK, NUM_HEADS * GRP], f32, name=f"ctxU{g}",
                                   tag=f"ctxU{g}") for g in range(NG)]
                ctxH = [upool.tile([HEAD_DIM, NUM_HEADS * GRP], f32r, name=f"ctxH{i}",
                                   tag=f"ctxH{i}") for i in range(NG)]

                def attention_group(g, h):
                    ctx_ps = apsum.tile([VBLK, GRP], f32, tag="ctx", name="ctx_ps")
                    for c, (k0, k1, w0, w1, mi) in enumerate(CHUNKS):
                        ksz = k1 - k0
                        win = w1 - w0
                        s_ps = apsum.tile([128, GRP], f32, tag="s", name="s_ps",
                                          bufs=4)
                        nc.tensor.matmul(
                            s_ps[0:ksz, :],
                            kTh[h][:, GRP * g + k0: GRP * g + k1],
                            qTh[h][:, GRP * g: GRP * (g + 1)],
                            start=True, stop=True)
                        ex = apool.tile([128, 192], bf16, tag="ex", name="ex",
                                        bufs=6)
                        nc.scalar.activation(
                            ex[0:ksz, 0:win], s_ps[0:ksz, w0:w1],
                            mybir.ActivationFunctionType.Exp)
                        nc.vector.tensor_tensor(
                            out=ex[0:ksz, 0:win], in0=ex[0:ksz, 0:win],
                            in1=masks[mi][0:ksz, :], op=mybir.AluOpType.mult)
                        nc.tensor.matmul(
                            ctx_ps[:, w0:w1],
                            vt[4 * g + c][0:ksz, h * VBLK:(h + 1) * VBLK],
                            ex[0:ksz, 0:win],
                            start=(c == 0), stop=(c == len(CHUNKS) - 1),
                            skip_group_check=True)
                    nc.vector.tensor_copy(
                        ctxU[g][:, h * GRP:(h + 1) * GRP], ctx_ps[:])

                def normalize_half(g):
                    # 1/denom = exp(-ln(d)) on ACT (DVE reciprocal is 7cyc/elem).
                    # Goes through a partition-0 tile: partition_broadcast
                    # replicates partition 0, not the AP's partition offset.
                    rl = apool.tile([1, NUM_HEADS * GRP], f32, tag="rl",
                                    name="rl", bufs=1)
                    nc.scalar.activation(rl[:], ctxU[g][HEAD_DIM:VBLK, :],
                                         mybir.ActivationFunctionType.Ln)
                    nc.scalar.activation(rl[:], rl[:],
                                         mybir.ActivationFunctionType.Exp, scale=-1.0)
                    # per-head broadcast+multiply so the out-projection's
                    # h-inner accumulation can start as soon as head 0 is done
                    for h in range(NUM_HEADS):
                        rdb = apool.tile([HEAD_DIM, GRP], f32,
                                         tag="rdb", name="rdb", bufs=2)
                        nc.gpsimd.partition_broadcast(
                            rdb[:], rl[:, h * GRP:(h + 1) * GRP])
                        nc.vector.tensor_tensor(
                            out=ctxH[g][:, h * GRP:(h + 1) * GRP],
                            in0=ctxU[g][0:HEAD_DIM, h * GRP:(h + 1) * GRP],
                            in1=rdb[:], op=mybir.AluOpType.mult)

                def outproj_half(i):
                    for dc in range(KC):
                        op = opsum.tile([128, 512], f32, tag="po", name="ps_o")
                        for h in range(NUM_HEADS):
                            nc.tensor.matmul(
                                op[:],
                                wo[:, h * D + dc * 128: h * D + dc * 128 + 128],
                                ctxH[i][:, h * GRP:(h + 1) * GRP],
                                start=(h == 0), stop=(h == NUM_HEADS - 1))
                        ost = opool.tile([128, 512], f32, tag="ost", name="ost")
                        nc.vector.tensor_scalar_add(ost[:], op[:], bo2t[:, dc:dc + 1])
                        nc.sync.dma_start(
                            out.ap()[dc * 128:(dc + 1) * 128, i * 512:(i + 1) * 512],
                            ost[:])

                for g in range(NG):
                    for h in range(NUM_HEADS):
                        attention_group(g, h)
                    normalize_half(g)
                    outproj_half(g)
    nc.compile()
    return nc


_program_cache = {}


def _get_program():
    if "nc" not in _program_cache:
        _program_cache["nc"] = _build_program()
    return _program_cache["nc"]


def _host_masks():
    # Three mask patterns: d = key - token offset within the chunk window.
    # m0 (first chunk): d = kk - mm; m1/m2 (later chunks): d = kk - mm + 64.
    import ml_dtypes
    masks = []
    for (nk, nw, off) in ((128, 128, 0), (128, 192, HALO), (64, 64, HALO)):
        kk, mm = np.meshgrid(np.arange(nk), np.arange(nw), indexing="ij")
        d = kk - mm + off
        valid = (d >= 0) & (d <= HALO) & (d % 4 == 0) & (d != OVERLAP)
        masks.append(valid.astype(ml_dtypes.bfloat16))
    return masks


def kernel(main, begin, end, in_proj_w, in_proj_b, out_proj_w, out_proj_b):
    import ml_dtypes
    from concourse.bass_utils import run_bass_kernel_spmd

    main = np.asarray(main, np.float32)
    begin = np.asarray(begin, np.float32)
    end = np.asarray(end, np.float32)
    in_proj_w = np.asarray(in_proj_w, np.float32)
    in_proj_b = np.asarray(in_proj_b, np.float32)
    out_proj_w = np.asarray(out_proj_w, np.float32)
    out_proj_b = np.asarray(out_proj_b, np.float32)

    D = EMBED_DIM
    scale = HEAD_DIM ** -0.5
    wq, wk, wv = in_proj_w[:D], in_proj_w[D:2 * D], in_proj_w[2 * D:]
    bq_, bv = in_proj_b[:D], in_proj_b[2 * D:3 * D]
    combined = np.concatenate([begin, main, end], axis=0)  # [N + 64, D]

    wqT = np.ascontiguousarray(wq.T * scale)
    wkT = np.ascontiguousarray(wk.T)
    wvT = np.ascontiguousarray(wv.T).astype(ml_dtypes.bfloat16)
    woT = np.ascontiguousarray(out_proj_w.T)
    bq_heads = np.ascontiguousarray((bq_ * scale).reshape(NUM_HEADS, HEAD_DIM).T)
    bo2 = out_proj_w @ bv + out_proj_b                      # [768]
    bo2_chunks = np.ascontiguousarray(bo2.reshape(KC, 128).T)
    masks = _host_masks()

    shared = {
        "wqT": wqT, "wkT": wkT, "wvT": wvT, "woT": woT,
        "bq": bq_heads, "bo2": bo2_chunks,
        "m0": masks[0], "m1": masks[1], "m2": masks[2],
    }
    in_maps = []
    for c in range(N_CORES):
        xTc = np.ascontiguousarray(combined[c * TOK: c * TOK + ROWS].T)
        in_maps.append({**shared, "xT": xTc})

    nc = _get_program()
    res = run_bass_kernel_spmd(nc, in_maps, core_ids=list(range(N_CORES)),
                               **_program_cache.get("run_kwargs", {}))
    _program_cache["last_result"] = res

    outp = np.empty((N_LINES, 2 * D), np.float32)
    outp[:, :D] = main
    for c in range(N_CORES):
        outp[c * TOK:(c + 1) * TOK, D:] = res.results[c]["out"].T
    return outp
